# revision 23
# baseline (speedup 1.0000x reference)
"""Trainium2 Bass kernel for nn_CapsRoutingLayer (capsule dynamic routing).

Sharding: data-parallel over batch. 8 NeuronCores, 8 batch elements each.
Each core streams the full (host bf16-cast, chunk-major) W once, builds
x_hat in SBUF (bf16) with one full 128x128 block-diagonal PE matmul per
16-capsule chunk, and runs the 3 routing iterations on-core.

Routing uses the cumulative-coupling identity: the logits after t updates
are b_t = (sum_{tau<t} v_tau) . x_hat, so no logits tensor is stored; each
iteration recomputes z = V.x_hat fresh (V accumulated in f32).

Per-iteration window pipeline (8 chunks per window, 16 windows):
  DVE  z-mult   y2 = x_hat (.) vrep          (bf16 2x mode)
  Pool tree l1/l2 (+ DVE l3/l4): z = sum_d y2
  ACT  exp      e = exp(z)   (single act table: ln/exp/copy/square)
  Pool sum_o + normalize -> c
  DVE  s-mult   y = c (.) x_hat
  PE   fold     s += pfold^T @ y  (PSUM f32 accumulation over all n)
The sum-over-n lives entirely on the TensorEngine (stationary pfold lhsT),
and squash's sqrt is exp(0.5*ln(u)) to keep one activation table loaded.

Self-contained: hardcodes all shapes from the problem spec.
  x: (64, 2048, 8) f32;  W: (2048, 32, 16, 8) f32  ->  v: (64, 32, 16) f32
"""

import sys

sys.path.insert(0, "/opt/trn_rl_repo")

import numpy as np
import ml_dtypes

# ---- problem sizes (hardcoded) ----
B_FULL, N, O, D, I = 64, 2048, 32, 16, 8
NCORES = 8
B = B_FULL // NCORES  # 8 batch elements per core
DO = D * O  # 512, on-chip innermost layout is (d, o)
NCH = N // 16  # 128 chunks of 16 capsules
WCH = 8  # chunks per routing window
NW = NCH // WCH  # 16 windows
N_ROUTING = 3

_NC = None


def _emit(tc, dram):
    import concourse.bass as bass
    from concourse import mybir

    nc = tc.nc
    BF = mybir.dt.bfloat16
    F32 = mybir.dt.float32
    AX = mybir.AxisListType
    ALU = mybir.AluOpType
    ACTF = mybir.ActivationFunctionType

    wt_d, xb_d, pfold_d, rep8_d, out_d = (
        dram["wt"], dram["xb"], dram["pfold"], dram["rep8"], dram["out"],
    )

    from contextlib import ExitStack

    ctx = ExitStack()
    const = ctx.enter_context(tc.tile_pool(name="const", bufs=1))
    persist = ctx.enter_context(tc.tile_pool(name="persist", bufs=1))
    scratch = ctx.enter_context(tc.tile_pool(name="scratch", bufs=1))

    # ---- constants ----
    pfold = const.tile([128, 8], BF)
    nc.sync.dma_start(out=pfold[:], in_=pfold_d[:])
    rep8 = const.tile([8, 128], BF)
    nc.sync.dma_start(out=rep8[:], in_=rep8_d[:])

    # ---- persistent tensors ----
    xh = persist.tile([128, NCH, D, O], BF)  # x_hat, 128KB/partition
    cbuf = persist.tile([128, NCH, O], BF)  # exp(z) -> c, per iteration
    esum = persist.tile([128, NCH], F32)  # softmax denominators
    vrep = persist.tile([128, DO], BF)  # V replicated to all partitions
    # squash / V smalls (8 partitions)
    Vc = scratch.tile([8, DO], F32, tag="Vc")  # cumulative V (f32)
    vsb = scratch.tile([8, DO], BF, tag="vsb")
    ssb = scratch.tile([8, DO], F32, tag="ssb")
    ssq = scratch.tile([8, DO], F32, tag="ssq")
    sq1 = scratch.tile([8, 8, O], F32, tag="sq1")
    sq2 = scratch.tile([8, 4, O], F32, tag="sq2")
    sq3 = scratch.tile([8, 2, O], F32, tag="sq3")
    n2 = scratch.tile([8, O], F32, tag="n2")
    lnn = scratch.tile([8, O], F32, tag="lnn")
    nr = scratch.tile([8, O], F32, tag="nr")
    den = scratch.tile([8, O], F32, tag="den")
    fac = scratch.tile([8, O], F32, tag="fac")
    vout = scratch.tile([8, DO], F32, tag="vout")

    with (
        tc.tile_pool(name="s0ps_pool", bufs=1, space="PSUM") as s0pool,
        tc.tile_pool(name="routps", bufs=2, space="PSUM") as rps,
        tc.tile_pool(name="vps_pool", bufs=1, space="PSUM") as vpool,
    ):
        s0ps = s0pool.tile([8, DO], F32)

        # ---- phase 1: build x_hat + s0 (software-pipelined) ----
        # Per chunk-pair P: PE matmuls into a 2-bank PSUM tile, one fused
        # copy to SBUF (engines rotate), and PE s0-fold matmuls for pair
        # P-2 (lag keeps PE streaming while copies drain).
        with (
            tc.tile_pool(name="wpool", bufs=2) as wpool,
            tc.tile_pool(name="xbpool", bufs=2) as xbpool,
            tc.tile_pool(name="buildps", bufs=2, space="PSUM") as bps,
        ):
            def s0_fold(cc):
                nc.tensor.matmul(
                    s0ps[:], pfold[:],
                    xh[:, cc, :, :].rearrange("p d o -> p (d o)"),
                    start=(cc == 0), stop=(cc == NCH - 1),
                    skip_group_check=True,
                )

            wstep = xbg = ps2 = None
            for P in range(NCH // 2):
                for ch in (2 * P, 2 * P + 1):
                    gw, kw = ch // WCH, ch % WCH  # wt group of 8
                    gx, kx = ch // 16, ch % 16  # xb group of 16
                    if kw == 0:
                        wstep = wpool.tile(
                            [128, WCH, DO], BF, tag="w", name=f"w_{gw}"
                        )
                        nc.sync.dma_start(
                            out=wstep[:],
                            in_=wt_d[:, WCH * gw : WCH * gw + WCH, :],
                        )
                    if kx == 0:
                        xbg = xbpool.tile(
                            [128, 16, 128], BF, tag="xb", name=f"xb_{gx}"
                        )
                        nc.gpsimd.dma_start(
                            out=xbg[:], in_=xb_d[:, 16 * gx : 16 * gx + 16, :]
                        )
                    if ch % 2 == 0:
                        ps2 = bps.tile([128, 2, DO], F32, tag="bps", name=f"ps_{ch}")
                    nc.tensor.matmul(
                        ps2[:, ch % 2, :], xbg[:, kx, :], wstep[:, kw, :],
                        start=True, stop=True, skip_group_check=True,
                    )
                dst = xh[:, 2 * P : 2 * P + 2, :, :]
                src = ps2[:].rearrange("p c (d o) -> p c d o", d=D)
                # GPSIMD cannot read PSUM: alternate copies DVE/ACT only
                if P % 2 == 0:
                    nc.vector.tensor_copy(dst, src)
                else:
                    nc.scalar.copy(dst, src)
                if P >= 2:
                    s0_fold(2 * (P - 2))
                    s0_fold(2 * (P - 2) + 1)
            for cc in range(NCH - 4, NCH):
                s0_fold(cc)

        def squash(sps, it, scale):
            """v = squash(scale * s); accumulate V; refresh vsb/vrep."""
            last = it == N_ROUTING - 1
            nc.vector.tensor_copy(ssb[:], sps[:])
            nc.vector.tensor_mul(ssq[:], ssb[:], ssb[:])
            sv3 = ssq[:].rearrange("b (d o) -> b d o", d=D)
            nc.vector.tensor_add(sq1[:], sv3[:, 0:8, :], sv3[:, 8:16, :])
            nc.vector.tensor_add(sq2[:], sq1[:, 0:4, :], sq1[:, 4:8, :])
            nc.vector.tensor_add(sq3[:], sq2[:, 0:2, :], sq2[:, 2:4, :])
            nc.vector.tensor_add(n2[:], sq3[:, 0, :], sq3[:, 1, :])
            if scale != 1.0:
                nc.vector.tensor_scalar_mul(n2[:], n2[:], scale * scale)
            # sqrt(u) = exp(0.5*ln(u)): stays on the ln/exp activation table
            nc.scalar.activation(lnn[:], n2[:], ACTF.Ln)
            nc.scalar.activation(nr[:], lnn[:], ACTF.Exp, scale=0.5)
            nc.vector.tensor_scalar_add(den[:], n2[:], 1.0)
            nc.vector.reciprocal(den[:], den[:])
            nc.vector.tensor_mul(fac[:], den[:], nr[:])
            if scale != 1.0:
                nc.vector.tensor_scalar_mul(fac[:], fac[:], scale)
            fb = fac[:].unsqueeze(1).broadcast_to([8, D, O])
            sv = ssb[:].rearrange("b (d o) -> b d o", d=D)
            if last:
                nc.vector.tensor_mul(
                    vout[:].rearrange("b (d o) -> b d o", d=D), sv, fb
                )
                nc.sync.dma_start(out=out_d[:], in_=vout[:])
                return
            # V += v (f32), then vsb/vrep from cumulative V
            if it == 0:
                nc.vector.tensor_mul(Vc[:].rearrange("b (d o) -> b d o", d=D), sv, fb)
            else:
                nc.vector.tensor_mul(vout[:].rearrange("b (d o) -> b d o", d=D), sv, fb)
                nc.vector.tensor_add(Vc[:], Vc[:], vout[:])
            nc.vector.tensor_copy(vsb[:], Vc[:])
            vps = vpool.tile([128, DO], F32, tag="vps", name=f"vps_{it}")
            nc.tensor.matmul(vps[:], rep8[:], vsb[:], start=True, stop=True)
            nc.vector.tensor_copy(vrep[:], vps[:])

        # ---- iteration 0: uniform c -> s0 (already on PE) ----
        squash(s0ps, 0, scale=1.0 / O)

        # ---- iterations 1, 2 (window-pipelined) ----
        # Emit the z-mult of window w+1 before the dependent tail of window
        # w so every engine's in-order stream always has ready work.
        with (
            tc.tile_pool(name="y2pool", bufs=2) as y2pool,
            tc.tile_pool(name="ypool", bufs=2) as ypool,
            tc.tile_pool(name="tpool", bufs=2) as tpool,
        ):
            for it in range(1, N_ROUTING):
                sacc = rps.tile([8, DO], F32, tag="sacc", name=f"sacc_{it}")

                def zmult(w, it=it):
                    sl = slice(WCH * w, WCH * w + WCH)
                    y2 = y2pool.tile(
                        [128, WCH, DO], BF, tag="y2", name=f"y2_{it}_{w}"
                    )
                    y2v = y2[:].rearrange("p c (d o) -> p c d o", d=D)
                    vb = (
                        vrep[:].rearrange("p (d o) -> p d o", d=D)
                        .unsqueeze(1).broadcast_to([128, WCH, D, O])
                    )
                    nc.vector.tensor_mul(y2v, xh[:, sl, :, :], vb)
                    return y2

                def tail(w, y2, it=it, sacc=sacc):
                    sl = slice(WCH * w, WCH * w + WCH)
                    xv = xh[:, sl, :, :]
                    y2v = y2[:].rearrange("p c (d o) -> p c d o", d=D)
                    # d-reduction tree -> z (bf16)
                    l1 = tpool.tile([128, WCH, 8, O], BF, tag="l1", name=f"l1_{it}_{w}")
                    nc.gpsimd.tensor_add(l1[:], y2v[:, :, 0:8, :], y2v[:, :, 8:16, :])
                    l2 = tpool.tile([128, WCH, 4, O], BF, tag="l2", name=f"l2_{it}_{w}")
                    nc.gpsimd.tensor_add(l2[:], l1[:, :, 0:4, :], l1[:, :, 4:8, :])
                    l3 = tpool.tile([128, WCH, 2, O], BF, tag="l3", name=f"l3_{it}_{w}")
                    nc.gpsimd.tensor_add(l3[:], l2[:, :, 0:2, :], l2[:, :, 2:4, :])
                    zt = tpool.tile([128, WCH, O], BF, tag="zt", name=f"zt_{it}_{w}")
                    nc.vector.tensor_add(zt[:], l3[:, :, 0, :], l3[:, :, 1, :])
                    # softmax over o (logits bounded; skip max-subtract)
                    nc.scalar.activation(cbuf[:, sl, :], zt[:], ACTF.Exp)
                    nc.vector.tensor_reduce(
                        esum[:, sl], cbuf[:, sl, :], axis=AX.X, op=ALU.add
                    )
                    nc.vector.reciprocal(esum[:, sl], esum[:, sl])
                    eb = esum[:, sl].unsqueeze(2).broadcast_to([128, WCH, O])
                    nc.gpsimd.tensor_mul(cbuf[:, sl, :], cbuf[:, sl, :], eb)
                    # s-mult: y = c (.) xh
                    y = ypool.tile([128, WCH, DO], BF, tag="y", name=f"y_{it}_{w}")
                    yv = y[:].rearrange("p c (d o) -> p c d o", d=D)
                    cb = cbuf[:, sl, :].unsqueeze(2).broadcast_to([128, WCH, D, O])
                    # every 4th window's s-mult runs on Pool: its output only
                    # feeds the PE folds, so it is off the next window's
                    # DVE critical path
                    s_eng = nc.gpsimd if w % 4 == 1 else nc.vector
                    s_eng.tensor_mul(yv, xv, cb)
                    # fold over n on PE: s += pfold^T @ y
                    for k in range(WCH):
                        nc.tensor.matmul(
                            sacc[:], pfold[:], y[:, k, :],
                            start=(w == 0 and k == 0),
                            stop=(w == NW - 1 and k == WCH - 1),
                        )

                prev = None
                for w in range(NW):
                    y2 = zmult(w)
                    if prev is not None:
                        tail(w - 1, prev)
                    prev = y2
                tail(NW - 1, prev)
                squash(sacc, it, scale=1.0)

    ctx.close()


def build_nc():
    import concourse.bass as bass
    import concourse.tile as tile
    from concourse import bacc, mybir

    BF = mybir.dt.bfloat16
    F32 = mybir.dt.float32
    nc = bacc.Bacc(
        "TRN2",
        target_bir_lowering=False,
        debug=False,
        enable_asserts=False,
        num_devices=NCORES,
    )
    dram = {
        "wt": nc.dram_tensor("wt", [128, NCH, DO], BF, kind="ExternalInput").ap(),
        "xb": nc.dram_tensor("xb", [128, NCH, 128], BF, kind="ExternalInput").ap(),
        "pfold": nc.dram_tensor("pfold", [128, 8], BF, kind="ExternalInput").ap(),
        "rep8": nc.dram_tensor("rep8", [8, 128], BF, kind="ExternalInput").ap(),
        "out": nc.dram_tensor("out", [B, DO], F32, kind="ExternalOutput").ap(),
    }
    with tile.TileContext(nc) as tc:
        _emit(tc, dram)
    nc.compile()
    return nc


def make_host_inputs(x, W):
    """Host-side layout prep. Returns per-core in_maps."""
    bf = ml_dtypes.bfloat16
    x = np.asarray(x, np.float32)
    W = np.asarray(W, np.float32)
    # W (N, O, D, I) -> rows (nh, i), chunk-major, cols od=(d, o)
    # wt[nh*8+i, ch, d*O+o] = W[16*ch+nh, o, d, i]
    wt = (
        W.transpose(0, 3, 2, 1)  # (N, I, D, O)
        .reshape(NCH, 16, I, DO)  # (ch, nh, i, od)
        .transpose(1, 2, 0, 3)  # (nh, i, ch, od)
        .reshape(128, NCH, DO)
        .astype(bf)
    )
    wt = np.ascontiguousarray(wt)
    pfold = ((np.arange(128)[:, None] % 8) == np.arange(8)[None, :]).astype(bf)
    rep8 = (np.arange(8)[:, None] == (np.arange(128)[None, :] % 8)).astype(bf)
    in_maps = []
    for k in range(NCORES):
        xc = x[B * k : B * k + B]  # (B, N, I)
        # block-diagonal lhsT: xb[nh*8+i, ch, nh*8+b] = xc[b, 16*ch+nh, i]
        x4 = (
            xc.transpose(1, 2, 0)  # (N, I, B)
            .reshape(NCH, 16, I, B)  # (ch, nh, i, b)
            .transpose(1, 2, 0, 3)  # (nh, i, ch, b)
        )
        xb = np.zeros((16, I, NCH, 16, I), np.float32)
        for nh in range(16):
            xb[nh, :, :, nh, :] = x4[nh]
        xb = np.ascontiguousarray(xb.reshape(128, NCH, 128).astype(bf))
        in_maps.append({"wt": wt, "xb": xb, "pfold": pfold, "rep8": rep8})
    return in_maps


def assemble_out(core_outs):
    """core_outs[k]: (B, DO) f32 in (d, o) layout -> (64, O, D) f32."""
    outs = [
        np.asarray(o, np.float32).reshape(B, D, O).transpose(0, 2, 1)
        for o in core_outs
    ]
    return np.ascontiguousarray(np.concatenate(outs, axis=0))


def run(x, W, trace=False):
    """Build (cached), execute on 8 cores, return (out, exec_time_ns)."""
    global _NC
    from concourse.bass_utils import run_bass_kernel_spmd

    if _NC is None:
        _NC = build_nc()
    in_maps = make_host_inputs(x, W)
    res = run_bass_kernel_spmd(
        _NC, in_maps, core_ids=list(range(NCORES)), trace=trace
    )
    out = assemble_out([res.results[k]["out"] for k in range(NCORES)])
    return out, res.exec_time_ns


def kernel(x, W):
    import time

    for attempt in range(3):
        try:
            out, _ = run(x, W, trace=False)
            return out
        except Exception:
            if attempt == 2:
                raise
            time.sleep(2.0)


def _make_sharded_exe(x, W):
    """Compile the NEFF, jit the 8-core dispatch, device-put inputs.

    Returns (exe, fetch): exe() queues one full 8-core execution and
    returns the jax output arrays (async); fetch(arrs) -> (64, 32, 16).
    """
    global _NC
    import jax
    from jax.sharding import Mesh, PartitionSpec, NamedSharding
    from jax.experimental.shard_map import shard_map
    from concourse import mybir
    from concourse.bass2jax import (
        _bass_exec_p,
        install_neuronx_cc_hook,
        partition_id_tensor,
    )

    if _NC is None:
        _NC = build_nc()
    nc = _NC
    install_neuronx_cc_hook()
    in_maps = make_host_inputs(x, W)
    n_cores = NCORES

    in_names, out_names, out_avals, zero_outs = [], [], [], []
    partition_name = nc.partition_id_tensor.name if nc.partition_id_tensor else None
    for alloc in nc.m.functions[0].allocations:
        if not isinstance(alloc, mybir.MemoryLocationSet):
            continue
        name = alloc.memorylocations[0].name
        if alloc.kind == "ExternalInput":
            if name != partition_name:
                in_names.append(name)
        elif alloc.kind == "ExternalOutput":
            shape = list(alloc.tensor_shape)
            dt = mybir.dt.np(alloc.dtype)
            out_avals.append(jax.core.ShapedArray(shape, dt))
            out_names.append(name)
            zero_outs.append(np.zeros(shape, dt))
    n_params = len(in_names)
    n_outs = len(out_names)
    all_in_names = list(in_names) + out_names
    if partition_name is not None:
        all_in_names.append(partition_name)

    def _body(*args):
        operands = list(args)
        if partition_name is not None:
            operands.append(partition_id_tensor())
        outs = _bass_exec_p.bind(
            *operands,
            out_avals=tuple(out_avals),
            in_names=tuple(all_in_names),
            out_names=tuple(out_names),
            lowering_input_output_aliases=(),
            sim_require_finite=True,
            sim_require_nnan=True,
            nc=nc,
        )
        return tuple(outs)

    devices = jax.devices()[:n_cores]
    mesh = Mesh(np.asarray(devices), ("core",))
    in_specs = (PartitionSpec("core"),) * (n_params + n_outs)
    out_specs = (PartitionSpec("core"),) * n_outs
    sharded = jax.jit(
        shard_map(_body, mesh=mesh, in_specs=in_specs, out_specs=out_specs,
                  check_rep=False),
        keep_unused=True,
    )
    shard = NamedSharding(mesh, PartitionSpec("core"))
    concat_in = [
        jax.device_put(
            np.concatenate([np.asarray(in_maps[c][nm]) for c in range(n_cores)], 0),
            shard,
        )
        for nm in in_names
    ]
    concat_zeros = [
        jax.device_put(
            np.zeros((n_cores * z.shape[0], *z.shape[1:]), z.dtype), shard
        )
        for z in zero_outs
    ]

    def exe():
        return sharded(*concat_in, *concat_zeros)

    def fetch(out_arrs):
        outs = [
            np.asarray(out_arrs[0]).reshape(n_cores, *out_avals[0].shape)[c]
            for c in range(n_cores)
        ]
        return assemble_out(outs)

    return exe, fetch


def bench_setup(x, W):
    """Build + verify once; return (out, exe) for throughput timing."""
    import jax

    exe, fetch = _make_sharded_exe(x, W)
    arrs = exe()
    jax.block_until_ready(arrs)
    return fetch(arrs), exe


def bench_hw(x, W, iters=30):
    """Legacy synchronous per-call bench (kept for comparison)."""
    import time
    import jax

    exe, fetch = _make_sharded_exe(x, W)
    times = []
    arrs = None
    for i in range(iters):
        t0 = time.perf_counter()
        arrs = exe()
        jax.block_until_ready(arrs)
        times.append(time.perf_counter() - t0)
    return fetch(arrs), times


# revision 24
# speedup vs baseline: 1.1618x; 1.1618x over previous
"""Trainium2 Bass kernel for nn_CapsRoutingLayer (capsule dynamic routing).

Sharding: data-parallel over batch. 8 NeuronCores, 8 batch elements each.
Each core streams the full (host bf16-cast, chunk-major) W once, builds
x_hat in SBUF (bf16) with one full 128x128 block-diagonal PE matmul per
16-capsule chunk, and runs the 3 routing iterations on-core.

Routing uses the cumulative-coupling identity: the logits after t updates
are b_t = (sum_{tau<t} v_tau) . x_hat, so no logits tensor is stored; each
iteration recomputes z = V.x_hat fresh (V accumulated in f32).

Per-iteration window pipeline (8 chunks per window, 16 windows):
  DVE  z-mult   y2 = x_hat (.) vrep          (bf16 2x mode)
  Pool tree l1/l2 (+ DVE l3/l4): z = sum_d y2
  ACT  exp      e = exp(z)   (single act table: ln/exp/copy/square)
  Pool sum_o + normalize -> c
  DVE  s-mult   y = c (.) x_hat
  PE   fold     s += pfold^T @ y  (PSUM f32 accumulation over all n)
The sum-over-n lives entirely on the TensorEngine (stationary pfold lhsT),
and squash's sqrt is exp(0.5*ln(u)) to keep one activation table loaded.

Self-contained: hardcodes all shapes from the problem spec.
  x: (64, 2048, 8) f32;  W: (2048, 32, 16, 8) f32  ->  v: (64, 32, 16) f32
"""

import sys

sys.path.insert(0, "/opt/trn_rl_repo")

import numpy as np
import ml_dtypes

# ---- problem sizes (hardcoded) ----
B_FULL, N, O, D, I = 64, 2048, 32, 16, 8
NCORES = 8
B = B_FULL // NCORES  # 8 batch elements per core
DO = D * O  # 512, on-chip innermost layout is (d, o)
NCH = N // 16  # 128 chunks of 16 capsules
WCH = 8  # chunks per routing window
NW = NCH // WCH  # 16 windows
N_ROUTING = 3

_NC = None


def _emit(tc, dram):
    import concourse.bass as bass
    from concourse import mybir

    nc = tc.nc
    BF = mybir.dt.bfloat16
    F32 = mybir.dt.float32
    AX = mybir.AxisListType
    ALU = mybir.AluOpType
    ACTF = mybir.ActivationFunctionType

    wt_d, xb_d, pfold_d, rep8_d, out_d = (
        dram["wt"], dram["xb"], dram["pfold"], dram["rep8"], dram["out"],
    )

    from contextlib import ExitStack

    ctx = ExitStack()
    const = ctx.enter_context(tc.tile_pool(name="const", bufs=1))
    persist = ctx.enter_context(tc.tile_pool(name="persist", bufs=1))
    scratch = ctx.enter_context(tc.tile_pool(name="scratch", bufs=1))

    # ---- constants ----
    pfold = const.tile([128, 8], BF)
    nc.sync.dma_start(out=pfold[:], in_=pfold_d[:])
    rep8 = const.tile([8, 128], BF)
    nc.sync.dma_start(out=rep8[:], in_=rep8_d[:])

    # ---- persistent tensors ----
    xh = persist.tile([128, NCH, D, O], BF)  # x_hat, 128KB/partition
    cbuf = persist.tile([128, NCH, O], BF)  # exp(z) -> c, per iteration
    esum = persist.tile([128, NCH], F32)  # softmax denominators
    vrep = persist.tile([128, DO], BF)  # V replicated to all partitions
    # squash / V smalls (8 partitions)
    Vc = scratch.tile([8, DO], F32, tag="Vc")  # cumulative V (f32)
    vsb = scratch.tile([8, DO], BF, tag="vsb")
    ssb = scratch.tile([8, DO], F32, tag="ssb")
    ssq = scratch.tile([8, DO], F32, tag="ssq")
    sq1 = scratch.tile([8, 8, O], F32, tag="sq1")
    sq2 = scratch.tile([8, 4, O], F32, tag="sq2")
    sq3 = scratch.tile([8, 2, O], F32, tag="sq3")
    n2 = scratch.tile([8, O], F32, tag="n2")
    lnn = scratch.tile([8, O], F32, tag="lnn")
    nr = scratch.tile([8, O], F32, tag="nr")
    den = scratch.tile([8, O], F32, tag="den")
    fac = scratch.tile([8, O], F32, tag="fac")
    vout = scratch.tile([8, DO], F32, tag="vout")

    with (
        tc.tile_pool(name="s0ps_pool", bufs=1, space="PSUM") as s0pool,
        tc.tile_pool(name="routps", bufs=2, space="PSUM") as rps,
        tc.tile_pool(name="vps_pool", bufs=1, space="PSUM") as vpool,
    ):
        s0ps = s0pool.tile([8, DO], F32)

        # ---- phase 1: build x_hat + s0 (software-pipelined) ----
        # Per chunk-pair P: PE matmuls into a 2-bank PSUM tile, one fused
        # copy to SBUF (engines rotate), and PE s0-fold matmuls for pair
        # P-2 (lag keeps PE streaming while copies drain).
        with (
            tc.tile_pool(name="wpool", bufs=2) as wpool,
            tc.tile_pool(name="xbpool", bufs=2) as xbpool,
            tc.tile_pool(name="buildps", bufs=2, space="PSUM") as bps,
        ):
            def s0_fold(cc):
                nc.tensor.matmul(
                    s0ps[:], pfold[:],
                    xh[:, cc, :, :].rearrange("p d o -> p (d o)"),
                    start=(cc == 0), stop=(cc == NCH - 1),
                    skip_group_check=True,
                )

            wstep = xbg = ps2 = None
            for P in range(NCH // 2):
                for ch in (2 * P, 2 * P + 1):
                    gw, kw = ch // WCH, ch % WCH  # wt group of 8
                    gx, kx = ch // 16, ch % 16  # xb group of 16
                    if kw == 0:
                        wstep = wpool.tile(
                            [128, WCH, DO], BF, tag="w", name=f"w_{gw}"
                        )
                        nc.sync.dma_start(
                            out=wstep[:],
                            in_=wt_d[:, WCH * gw : WCH * gw + WCH, :],
                        )
                    if kx == 0:
                        xbg = xbpool.tile(
                            [128, 16, 128], BF, tag="xb", name=f"xb_{gx}"
                        )
                        nc.gpsimd.dma_start(
                            out=xbg[:], in_=xb_d[:, 16 * gx : 16 * gx + 16, :]
                        )
                    if ch % 2 == 0:
                        ps2 = bps.tile([128, 2, DO], F32, tag="bps", name=f"ps_{ch}")
                    nc.tensor.matmul(
                        ps2[:, ch % 2, :], xbg[:, kx, :], wstep[:, kw, :],
                        start=True, stop=True, skip_group_check=True,
                    )
                dst = xh[:, 2 * P : 2 * P + 2, :, :]
                src = ps2[:].rearrange("p c (d o) -> p c d o", d=D)
                # GPSIMD cannot read PSUM: alternate copies DVE/ACT only
                if P % 2 == 0:
                    nc.vector.tensor_copy(dst, src)
                else:
                    nc.scalar.copy(dst, src)
                if P >= 2:
                    s0_fold(2 * (P - 2))
                    s0_fold(2 * (P - 2) + 1)
            for cc in range(NCH - 4, NCH):
                s0_fold(cc)

        def squash(sps, it, scale):
            """v = squash(scale * s); accumulate V; refresh vsb/vrep."""
            last = it == N_ROUTING - 1
            nc.vector.tensor_copy(ssb[:], sps[:])
            nc.vector.tensor_mul(ssq[:], ssb[:], ssb[:])
            sv3 = ssq[:].rearrange("b (d o) -> b d o", d=D)
            nc.vector.tensor_add(sq1[:], sv3[:, 0:8, :], sv3[:, 8:16, :])
            nc.vector.tensor_add(sq2[:], sq1[:, 0:4, :], sq1[:, 4:8, :])
            nc.vector.tensor_add(sq3[:], sq2[:, 0:2, :], sq2[:, 2:4, :])
            nc.vector.tensor_add(n2[:], sq3[:, 0, :], sq3[:, 1, :])
            if scale != 1.0:
                nc.vector.tensor_scalar_mul(n2[:], n2[:], scale * scale)
            # sqrt(u) = exp(0.5*ln(u)): stays on the ln/exp activation table
            nc.scalar.activation(lnn[:], n2[:], ACTF.Ln)
            nc.scalar.activation(nr[:], lnn[:], ACTF.Exp, scale=0.5)
            nc.vector.tensor_scalar_add(den[:], n2[:], 1.0)
            nc.vector.reciprocal(den[:], den[:])
            nc.vector.tensor_mul(fac[:], den[:], nr[:])
            if scale != 1.0:
                nc.vector.tensor_scalar_mul(fac[:], fac[:], scale)
            fb = fac[:].unsqueeze(1).broadcast_to([8, D, O])
            sv = ssb[:].rearrange("b (d o) -> b d o", d=D)
            if last:
                nc.vector.tensor_mul(
                    vout[:].rearrange("b (d o) -> b d o", d=D), sv, fb
                )
                nc.sync.dma_start(out=out_d[:], in_=vout[:])
                return
            # V += v (f32), then vsb/vrep from cumulative V
            if it == 0:
                nc.vector.tensor_mul(Vc[:].rearrange("b (d o) -> b d o", d=D), sv, fb)
            else:
                nc.vector.tensor_mul(vout[:].rearrange("b (d o) -> b d o", d=D), sv, fb)
                nc.vector.tensor_add(Vc[:], Vc[:], vout[:])
            nc.vector.tensor_copy(vsb[:], Vc[:])
            vps = vpool.tile([128, DO], F32, tag="vps", name=f"vps_{it}")
            nc.tensor.matmul(vps[:], rep8[:], vsb[:], start=True, stop=True)
            nc.vector.tensor_copy(vrep[:], vps[:])

        # ---- iteration 0: uniform c -> s0 (already on PE) ----
        squash(s0ps, 0, scale=1.0 / O)

        # ---- iterations 1, 2 (window-pipelined) ----
        # Emit the z-mult of window w+1 before the dependent tail of window
        # w so every engine's in-order stream always has ready work.
        with (
            tc.tile_pool(name="y2pool", bufs=2) as y2pool,
            tc.tile_pool(name="ypool", bufs=2) as ypool,
            tc.tile_pool(name="tpool", bufs=2) as tpool,
        ):
            for it in range(1, N_ROUTING):
                sacc = rps.tile([8, DO], F32, tag="sacc", name=f"sacc_{it}")

                def zmult(w, it=it):
                    sl = slice(WCH * w, WCH * w + WCH)
                    y2 = y2pool.tile(
                        [128, WCH, DO], BF, tag="y2", name=f"y2_{it}_{w}"
                    )
                    y2v = y2[:].rearrange("p c (d o) -> p c d o", d=D)
                    vb = (
                        vrep[:].rearrange("p (d o) -> p d o", d=D)
                        .unsqueeze(1).broadcast_to([128, WCH, D, O])
                    )
                    nc.vector.tensor_mul(y2v, xh[:, sl, :, :], vb)
                    return y2

                def tail(w, y2, it=it, sacc=sacc):
                    sl = slice(WCH * w, WCH * w + WCH)
                    xv = xh[:, sl, :, :]
                    y2v = y2[:].rearrange("p c (d o) -> p c d o", d=D)
                    # d-reduction tree -> z (bf16)
                    l1 = tpool.tile([128, WCH, 8, O], BF, tag="l1", name=f"l1_{it}_{w}")
                    nc.gpsimd.tensor_add(l1[:], y2v[:, :, 0:8, :], y2v[:, :, 8:16, :])
                    l2 = tpool.tile([128, WCH, 4, O], BF, tag="l2", name=f"l2_{it}_{w}")
                    nc.gpsimd.tensor_add(l2[:], l1[:, :, 0:4, :], l1[:, :, 4:8, :])
                    l3 = tpool.tile([128, WCH, 2, O], BF, tag="l3", name=f"l3_{it}_{w}")
                    nc.gpsimd.tensor_add(l3[:], l2[:, :, 0:2, :], l2[:, :, 2:4, :])
                    zt = tpool.tile([128, WCH, O], BF, tag="zt", name=f"zt_{it}_{w}")
                    nc.vector.tensor_add(zt[:], l3[:, :, 0, :], l3[:, :, 1, :])
                    # softmax over o (logits bounded; skip max-subtract)
                    nc.scalar.activation(cbuf[:, sl, :], zt[:], ACTF.Exp)
                    nc.vector.tensor_reduce(
                        esum[:, sl], cbuf[:, sl, :], axis=AX.X, op=ALU.add
                    )
                    nc.vector.reciprocal(esum[:, sl], esum[:, sl])
                    eb = esum[:, sl].unsqueeze(2).broadcast_to([128, WCH, O])
                    nc.gpsimd.tensor_mul(cbuf[:, sl, :], cbuf[:, sl, :], eb)
                    # s-mult: y = c (.) xh
                    y = ypool.tile([128, WCH, DO], BF, tag="y", name=f"y_{it}_{w}")
                    yv = y[:].rearrange("p c (d o) -> p c d o", d=D)
                    cb = cbuf[:, sl, :].unsqueeze(2).broadcast_to([128, WCH, D, O])
                    nc.vector.tensor_mul(yv, xv, cb)
                    # fold over n on PE: s += pfold^T @ y
                    for k in range(WCH):
                        nc.tensor.matmul(
                            sacc[:], pfold[:], y[:, k, :],
                            start=(w == 0 and k == 0),
                            stop=(w == NW - 1 and k == WCH - 1),
                        )

                prev = None
                for w in range(NW):
                    y2 = zmult(w)
                    if prev is not None:
                        tail(w - 1, prev)
                    prev = y2
                tail(NW - 1, prev)
                squash(sacc, it, scale=1.0)

    ctx.close()


def build_nc():
    import concourse.bass as bass
    import concourse.tile as tile
    from concourse import bacc, mybir

    BF = mybir.dt.bfloat16
    F32 = mybir.dt.float32
    nc = bacc.Bacc(
        "TRN2",
        target_bir_lowering=False,
        debug=False,
        enable_asserts=False,
        num_devices=NCORES,
    )
    dram = {
        "wt": nc.dram_tensor("wt", [128, NCH, DO], BF, kind="ExternalInput").ap(),
        "xb": nc.dram_tensor("xb", [128, NCH, 128], BF, kind="ExternalInput").ap(),
        "pfold": nc.dram_tensor("pfold", [128, 8], BF, kind="ExternalInput").ap(),
        "rep8": nc.dram_tensor("rep8", [8, 128], BF, kind="ExternalInput").ap(),
        "out": nc.dram_tensor("out", [B, DO], F32, kind="ExternalOutput").ap(),
    }
    with tile.TileContext(nc) as tc:
        _emit(tc, dram)
    nc.compile()
    return nc


def make_host_inputs(x, W):
    """Host-side layout prep. Returns per-core in_maps."""
    bf = ml_dtypes.bfloat16
    x = np.asarray(x, np.float32)
    W = np.asarray(W, np.float32)
    # W (N, O, D, I) -> rows (nh, i), chunk-major, cols od=(d, o)
    # wt[nh*8+i, ch, d*O+o] = W[16*ch+nh, o, d, i]
    wt = (
        W.transpose(0, 3, 2, 1)  # (N, I, D, O)
        .reshape(NCH, 16, I, DO)  # (ch, nh, i, od)
        .transpose(1, 2, 0, 3)  # (nh, i, ch, od)
        .reshape(128, NCH, DO)
        .astype(bf)
    )
    wt = np.ascontiguousarray(wt)
    pfold = ((np.arange(128)[:, None] % 8) == np.arange(8)[None, :]).astype(bf)
    rep8 = (np.arange(8)[:, None] == (np.arange(128)[None, :] % 8)).astype(bf)
    in_maps = []
    for k in range(NCORES):
        xc = x[B * k : B * k + B]  # (B, N, I)
        # block-diagonal lhsT: xb[nh*8+i, ch, nh*8+b] = xc[b, 16*ch+nh, i]
        x4 = (
            xc.transpose(1, 2, 0)  # (N, I, B)
            .reshape(NCH, 16, I, B)  # (ch, nh, i, b)
            .transpose(1, 2, 0, 3)  # (nh, i, ch, b)
        )
        xb = np.zeros((16, I, NCH, 16, I), np.float32)
        for nh in range(16):
            xb[nh, :, :, nh, :] = x4[nh]
        xb = np.ascontiguousarray(xb.reshape(128, NCH, 128).astype(bf))
        in_maps.append({"wt": wt, "xb": xb, "pfold": pfold, "rep8": rep8})
    return in_maps


def assemble_out(core_outs):
    """core_outs[k]: (B, DO) f32 in (d, o) layout -> (64, O, D) f32."""
    outs = [
        np.asarray(o, np.float32).reshape(B, D, O).transpose(0, 2, 1)
        for o in core_outs
    ]
    return np.ascontiguousarray(np.concatenate(outs, axis=0))


def run(x, W, trace=False):
    """Build (cached), execute on 8 cores, return (out, exec_time_ns)."""
    global _NC
    from concourse.bass_utils import run_bass_kernel_spmd

    if _NC is None:
        _NC = build_nc()
    in_maps = make_host_inputs(x, W)
    res = run_bass_kernel_spmd(
        _NC, in_maps, core_ids=list(range(NCORES)), trace=trace
    )
    out = assemble_out([res.results[k]["out"] for k in range(NCORES)])
    return out, res.exec_time_ns


def kernel(x, W):
    import time

    for attempt in range(3):
        try:
            out, _ = run(x, W, trace=False)
            return out
        except Exception:
            if attempt == 2:
                raise
            time.sleep(2.0)


def _make_sharded_exe(x, W):
    """Compile the NEFF, jit the 8-core dispatch, device-put inputs.

    Returns (exe, fetch): exe() queues one full 8-core execution and
    returns the jax output arrays (async); fetch(arrs) -> (64, 32, 16).
    """
    global _NC
    import jax
    from jax.sharding import Mesh, PartitionSpec, NamedSharding
    from jax.experimental.shard_map import shard_map
    from concourse import mybir
    from concourse.bass2jax import (
        _bass_exec_p,
        install_neuronx_cc_hook,
        partition_id_tensor,
    )

    if _NC is None:
        _NC = build_nc()
    nc = _NC
    install_neuronx_cc_hook()
    in_maps = make_host_inputs(x, W)
    n_cores = NCORES

    in_names, out_names, out_avals, zero_outs = [], [], [], []
    partition_name = nc.partition_id_tensor.name if nc.partition_id_tensor else None
    for alloc in nc.m.functions[0].allocations:
        if not isinstance(alloc, mybir.MemoryLocationSet):
            continue
        name = alloc.memorylocations[0].name
        if alloc.kind == "ExternalInput":
            if name != partition_name:
                in_names.append(name)
        elif alloc.kind == "ExternalOutput":
            shape = list(alloc.tensor_shape)
            dt = mybir.dt.np(alloc.dtype)
            out_avals.append(jax.core.ShapedArray(shape, dt))
            out_names.append(name)
            zero_outs.append(np.zeros(shape, dt))
    n_params = len(in_names)
    n_outs = len(out_names)
    all_in_names = list(in_names) + out_names
    if partition_name is not None:
        all_in_names.append(partition_name)

    def _body(*args):
        operands = list(args)
        if partition_name is not None:
            operands.append(partition_id_tensor())
        outs = _bass_exec_p.bind(
            *operands,
            out_avals=tuple(out_avals),
            in_names=tuple(all_in_names),
            out_names=tuple(out_names),
            lowering_input_output_aliases=(),
            sim_require_finite=True,
            sim_require_nnan=True,
            nc=nc,
        )
        return tuple(outs)

    devices = jax.devices()[:n_cores]
    mesh = Mesh(np.asarray(devices), ("core",))
    in_specs = (PartitionSpec("core"),) * (n_params + n_outs)
    out_specs = (PartitionSpec("core"),) * n_outs
    sharded = jax.jit(
        shard_map(_body, mesh=mesh, in_specs=in_specs, out_specs=out_specs,
                  check_rep=False),
        keep_unused=True,
    )
    shard = NamedSharding(mesh, PartitionSpec("core"))
    concat_in = [
        jax.device_put(
            np.concatenate([np.asarray(in_maps[c][nm]) for c in range(n_cores)], 0),
            shard,
        )
        for nm in in_names
    ]
    concat_zeros = [
        jax.device_put(
            np.zeros((n_cores * z.shape[0], *z.shape[1:]), z.dtype), shard
        )
        for z in zero_outs
    ]

    def exe():
        return sharded(*concat_in, *concat_zeros)

    def fetch(out_arrs):
        outs = [
            np.asarray(out_arrs[0]).reshape(n_cores, *out_avals[0].shape)[c]
            for c in range(n_cores)
        ]
        return assemble_out(outs)

    return exe, fetch


def bench_setup(x, W):
    """Build + verify once; return (out, exe) for throughput timing."""
    import jax

    exe, fetch = _make_sharded_exe(x, W)
    arrs = exe()
    jax.block_until_ready(arrs)
    return fetch(arrs), exe


def bench_hw(x, W, iters=30):
    """Legacy synchronous per-call bench (kept for comparison)."""
    import time
    import jax

    exe, fetch = _make_sharded_exe(x, W)
    times = []
    arrs = None
    for i in range(iters):
        t0 = time.perf_counter()
        arrs = exe()
        jax.block_until_ready(arrs)
        times.append(time.perf_counter() - t0)
    return fetch(arrs), times
